# revision 8
# baseline (speedup 1.0000x reference)
"""Trainium2 Bass kernel for nn_KernelAttention.

Math notes (derived from the reference):
  - softmax over a singleton axis == 1.0, so attention output == v_s and the
    q/k branches never affect `output`. attns is identically ones.
  - The torch-style views make per-core work fully local when sharding the
    batch (mb) 8 ways: core c needs only v rows [c*512, (c+1)*512), w_vs[c],
    and the (replicated) proj weight.
Per core c:
  outputs_cT[i*128+t, bb] = sum_d v[c*512+bb, i, d] * w_vs[c][d, t]
  proj[bb, o]  = sum_j outputs_c[bb, j] * proj_w.T[j, o]      (o in [0, 8192))
  z[bb, o]     = proj[bb, o] + proj_b[o] + v[c*512+bb, o//1024, o%1024]
  out row (bb, i) = LayerNorm_d(z[bb, i*1024:(i+1)*1024])  (unbiased std,
                    eps added OUTSIDE sqrt), then * gamma + beta.
proj_b is folded into the residual on the host; gamma/beta applied on the
host only if non-trivial (they are 1/0 in setup_inputs).
"""

import sys

import numpy as np

sys.path.insert(0, "/opt/trn_rl_repo")

MB, NK, DM, DT = 4096, 8, 1024, 128
NCORES = 8
RPC = MB // NCORES          # 512 batch rows per core
FR = RPC * NK               # 4096 flat (row, kernel) pairs per core
KD = DM // 128              # 8 contraction tiles
NO = 512                    # o-tile (psum free dim)
MBLK = RPC // 128           # 4 output partition blocks
EPS = 1e-3

_CACHE = {}


def _build_nc():
    import concourse.bacc as bacc
    import concourse.tile as tile
    from concourse import mybir

    f32 = mybir.dt.float32
    f32r = mybir.dt.float32r

    nc = bacc.Bacc("TRN2", target_bir_lowering=False)
    vtp = nc.dram_tensor("vtp", [DM, FR], f32r, kind="ExternalInput")
    vres = nc.dram_tensor("vres", [FR, DM], f32, kind="ExternalInput")
    wv = nc.dram_tensor("wv", [DM, DT], f32r, kind="ExternalInput")
    pwt = nc.dram_tensor("pwt", [DM, NK * DM], f32r, kind="ExternalInput")
    out = nc.dram_tensor("out", [RPC, NK * DM], f32, kind="ExternalOutput")

    # DRAM views
    # vres rows r = bb*8 + i with bb = m*128 + p  ->  [m, i, half, p, q]
    vres_r = vres.rearrange(
        "(m p i) (h q) -> m i h p q", m=MBLK, p=128, i=NK, h=2
    )
    # out rows bb = m*128 + p, cols o = i*1024 + d  ->  [m, i, p, d]
    out_r = out.rearrange("(m p) (i d) -> m i p d", m=MBLK, p=128, i=NK)

    with tile.TileContext(nc) as tc:
        with (
            tc.tile_pool(name="const", bufs=1) as const_pool,
            tc.tile_pool(name="vtp_pool", bufs=3) as vtp_pool,
            tc.tile_pool(name="outT_pool", bufs=1) as outT_pool,
            tc.tile_pool(name="pwt_pool", bufs=4) as pwt_pool,
            tc.tile_pool(name="z_pool", bufs=2) as z_pool,
            tc.tile_pool(name="vres_pool", bufs=6) as vres_pool,
            tc.tile_pool(name="y_pool", bufs=3) as y_pool,
            tc.tile_pool(name="stat_pool", bufs=4) as stat_pool,
            tc.tile_pool(name="ps1", bufs=2, space="PSUM") as ps1,
            tc.tile_pool(name="ps2", bufs=6, space="PSUM") as ps2,
        ):
            wv_sb = const_pool.tile([128, KD, DT], f32r)
            nc.sync.dma_start(
                out=wv_sb, in_=wv.rearrange("(kd p) t -> p kd t", p=128)
            )
            zero_col = const_pool.tile([128, 1], f32)
            nc.vector.memset(zero_col, 0.0)
            eps_col = const_pool.tile([128, 1], f32)
            nc.vector.memset(eps_col, EPS)

            # ---- stage 1: outT[:, i, :] = (v_s chunk i).T = w_v.T @ v_c[:, i, :].T
            outT = outT_pool.tile([128, KD, RPC], f32r)
            for i in range(NK):
                vt_sb = vtp_pool.tile([128, KD, RPC], f32r, tag="vt")
                nc.sync.dma_start(
                    out=vt_sb,
                    in_=vtp[:, i * RPC : (i + 1) * RPC].rearrange(
                        "(kd p) b -> p kd b", p=128
                    ),
                )
                ps = ps1.tile([128, RPC], f32, tag="ps1")
                for kd in range(KD):
                    nc.tensor.matmul(
                        ps,
                        wv_sb[:, kd, :],
                        vt_sb[:, kd, :],
                        start=(kd == 0),
                        stop=(kd == KD - 1),
                    )
                nc.vector.tensor_copy(out=outT[:, i, :], in_=ps)

            # ---- stage 2: proj + residual + LayerNorm, streamed over o
            for i_ln in range(NK):
                z_tiles = [
                    z_pool.tile([128, DM], f32, tag=f"z{m}", name=f"z{m}_{i_ln}")
                    for m in range(MBLK)
                ]
                for half in range(2):
                    ot = 2 * i_ln + half
                    pw_sb = pwt_pool.tile([128, KD, NO], f32r, tag="pw")
                    nc.sync.dma_start(
                        out=pw_sb,
                        in_=pwt[:, ot * NO : (ot + 1) * NO].rearrange(
                            "(kd p) o -> p kd o", p=128
                        ),
                    )
                    for m in range(MBLK):
                        ps = ps2.tile([128, NO], f32, tag="ps2")
                        for kd in range(KD):
                            nc.tensor.matmul(
                                ps,
                                outT[:, kd, m * 128 : (m + 1) * 128],
                                pw_sb[:, kd, :],
                                start=(kd == 0),
                                stop=(kd == KD - 1),
                            )
                        vr_sb = vres_pool.tile([128, NO], f32, tag="vr")
                        nc.sync.dma_start(out=vr_sb, in_=vres_r[m, i_ln, half])
                        nc.vector.tensor_add(
                            out=z_tiles[m][:, half * NO : (half + 1) * NO],
                            in0=ps,
                            in1=vr_sb,
                        )
                for m in range(MBLK):
                    z = z_tiles[m]
                    zr = z.rearrange("p (s q) -> p s q", s=2)
                    stats = stat_pool.tile([128, 2, 6], f32, tag="st")
                    for s in range(2):
                        nc.vector.bn_stats(out=stats[:, s, :], in_=zr[:, s, :])
                    mv = stat_pool.tile([128, 2], f32, tag="mv")
                    nc.vector.bn_aggr(out=mv, in_=stats)
                    # sigma = sqrt(var * n/(n-1)); inv = 1/(sigma + eps)
                    sig = stat_pool.tile([128, 1], f32, tag="sig")
                    nc.scalar.activation(
                        out=sig,
                        in_=mv[:, 1:2],
                        func=mybir.ActivationFunctionType.Sqrt,
                        bias=zero_col,
                        scale=float(DM) / float(DM - 1),
                    )
                    nc.vector.tensor_add(out=sig, in0=sig, in1=eps_col)
                    inv = stat_pool.tile([128, 1], f32, tag="inv")
                    nc.vector.reciprocal(out=inv, in_=sig)
                    y = y_pool.tile([128, DM], f32, tag="y")
                    nc.vector.tensor_scalar(
                        out=y,
                        in0=z,
                        scalar1=mv[:, 0:1],
                        scalar2=inv,
                        op0=mybir.AluOpType.subtract,
                        op1=mybir.AluOpType.mult,
                    )
                    nc.sync.dma_start(out=out_r[m, i_ln], in_=y)
    nc.compile()
    return nc


def kernel(q, k, v, w_qs, w_ks, w_vs, proj_w, proj_b, gamma, beta):
    from concourse.bass_utils import run_bass_kernel_spmd

    v = np.ascontiguousarray(np.asarray(v, dtype=np.float32))
    w_vs = np.asarray(w_vs, dtype=np.float32)
    proj_w = np.asarray(proj_w, dtype=np.float32)
    proj_b = np.asarray(proj_b, dtype=np.float32)
    gamma = np.asarray(gamma, dtype=np.float32)
    beta = np.asarray(beta, dtype=np.float32)

    if "nc" not in _CACHE:
        _CACHE["nc"] = _build_nc()
    nc = _CACHE["nc"]

    pwt = np.ascontiguousarray(proj_w.T)  # [1024, 8192]
    pb_fold = proj_b.reshape(NK, DM)

    in_maps = []
    for c in range(NCORES):
        vc = v[c * RPC : (c + 1) * RPC]  # [512, 8, 1024]
        vtp = np.ascontiguousarray(vc.transpose(2, 1, 0)).reshape(DM, FR)
        if proj_b.any():
            vres = (vc + pb_fold[None]).reshape(FR, DM)
        else:
            vres = vc.reshape(FR, DM)
        in_maps.append(
            {
                "vtp": vtp,
                "vres": np.ascontiguousarray(vres),
                "wv": np.ascontiguousarray(w_vs[c]),
                "pwt": pwt,
            }
        )

    res = run_bass_kernel_spmd(nc, in_maps, core_ids=list(range(NCORES)))
    output = np.concatenate(
        [r["out"].reshape(RPC, NK, DM) for r in res.results], axis=0
    )
    if not (np.all(gamma == 1.0) and np.all(beta == 0.0)):
        output = output * gamma + beta
    attns = np.ones((MB * NK, 1, 1), dtype=np.float32)
    return output.astype(np.float32), attns
